# revision 1
# baseline (speedup 1.0000x reference)
# Trainium2 Bass kernel for the ContextBlock problem.
#
# Reference computation (per sample b):
#   xc    = concat(x0..x3)            [C=1024, HW=4096]
#   attn  = softmax(wm @ xc)          [HW]
#   ctx   = xc @ attn                 [C]
#   mul   = residual-gated MLP stack (sigmoid branch)   [C]
#   add   = residual-gated MLP stack (linear branch)    [C]
#   out   = sum_l (x_l * mul_l + add_l)                 [CL=256, HW]
#
# Distribution: data-parallel over batch, one sample per NeuronCore (B=8).
# No collectives required.
#
# Per-core dataflow:
#   pass1: PE transposes x (bf16) into xT while accumulating logits in PSUM
#   softmax: row reduce + tiny PE transpose/broadcast matmuls + fused exp
#   pass2: context = attn.T @ xT on PE (contraction over HW on partitions)
#   gates: weight-stationary matmuls; LN stats via ones-matmuls
#   pass3: diag(mul) @ x accumulated over levels in PSUM, bias=add-sum

import numpy as np
import ml_dtypes
from contextlib import ExitStack

import concourse.bass as bass
import concourse.bacc as bacc
import concourse.mybir as mybir
import concourse.tile as tile

BF = mybir.dt.bfloat16
F32 = mybir.dt.float32
AF = mybir.ActivationFunctionType
ALU = mybir.AluOpType
AX = mybir.AxisListType

B, L, CL, H, W = 8, 4, 256, 64, 64
C = L * CL          # 1024
HW = H * W          # 4096
P = C // 4          # 256
R = 2
EPS = 1e-5
NJ = C // 128       # 8   c-slabs
NI = HW // 128      # 32  n-chunks
NCORES = 8

_CACHE = {}


def _build_nc():
    nc = bacc.Bacc()

    x_d = nc.dram_tensor("x", [C, HW], BF, kind="ExternalInput")
    rhsi_d = nc.dram_tensor("rhsi", [128, 128], BF, kind="ExternalInput")
    wmc_d = nc.dram_tensor("wmc", [128, NJ], BF, kind="ExternalInput")
    cf32_d = nc.dram_tensor("cf32", [128, 256], F32, kind="ExternalInput")
    wg1_d = nc.dram_tensor("wg1", [4, NJ, 128, 1024], BF, kind="ExternalInput")
    wg2_d = nc.dram_tensor("wg2", [4, 128, 2048], BF, kind="ExternalInput")
    sm_d = nc.dram_tensor("smalls", [128, 128], F32, kind="ExternalInput")
    out_d = nc.dram_tensor("out", [CL, HW], F32, kind="ExternalOutput")

    with tile.TileContext(nc) as tc, ExitStack() as ctx:
        resid = ctx.enter_context(tc.tile_pool(name="resid", bufs=1))
        wpool = ctx.enter_context(tc.tile_pool(name="wpool", bufs=6))
        w2pool = ctx.enter_context(tc.tile_pool(name="w2pool", bufs=2))
        spool = ctx.enter_context(tc.tile_pool(name="spool", bufs=1))
        stpool = ctx.enter_context(tc.tile_pool(name="stage", bufs=4))
        dpool = ctx.enter_context(tc.tile_pool(name="diag", bufs=1))

        # ---- resident tiles + loads ----
        x_sb = resid.tile([128, NJ, HW], BF, tag="x")
        rhsi = resid.tile([128, 128], BF, tag="rhsi")
        wmc = resid.tile([128, NJ], BF, tag="wmc")
        cf32 = resid.tile([128, 256], F32, tag="cf32")
        sm = resid.tile([128, 128], F32, tag="sm")

        nc.sync.dma_start(rhsi[:], rhsi_d[:])
        nc.sync.dma_start(wmc[:], wmc_d[:])
        nc.sync.dma_start(cf32[:], cf32_d[:])
        nc.sync.dma_start(sm[:], sm_d[:])
        for j in range(NJ):
            nc.sync.dma_start(x_sb[:, j, :], x_d[128 * j:128 * (j + 1), :])

        idf = cf32[:, 0:128]          # identity f32
        ones_col = cf32[:, 128:129]   # [128,1] ones
        ones_row = cf32[0:1, 128:256]  # [1,128] ones

        # ---- pass 1: logits row (wm stationary; rides the x DMA) ----
        attn_row = spool.tile([1, HW], F32, tag="attn_row")
        rowsum = spool.tile([1, 1], F32, tag="rowsum")
        with tc.tile_pool(name="psrow", bufs=1,
                          space=bass.MemorySpace.PSUM) as psrow:
            lg_row = psrow.tile([1, HW], F32, tag="row")
            for j in range(NJ):
                for nch in range(NJ):
                    nc.tensor.matmul(
                        lg_row[0:1, 512 * nch:512 * (nch + 1)],
                        wmc[:, j:j + 1],
                        x_sb[:, j, 512 * nch:512 * (nch + 1)],
                        start=(j == 0), stop=(j == NJ - 1),
                    )
            # exp + row-sum straight out of PSUM (|logits| < ~4: no max
            # subtraction needed; softmax is shift invariant)
            nc.scalar.activation(
                attn_row[:], lg_row[:], AF.Exp, accum_out=rowsum[:]
            )

        ps = ctx.enter_context(
            tc.tile_pool(name="ps", bufs=4, space=bass.MemorySpace.PSUM)
        )
        inv = spool.tile([1, 1], F32, tag="inv")
        nc.vector.reciprocal(inv[:], rowsum[:])
        ps_ib = ps.tile([128, 1], F32, tag="tiny", bufs=4)
        nc.tensor.matmul(ps_ib[:], ones_row, inv[:])
        inv_bc = spool.tile([128, 1], F32, tag="inv_bc")
        nc.vector.tensor_copy(inv_bc[:], ps_ib[:])

        # ---- attn broadcast over partitions: 8 ones-matmuls ----
        attn_bc = resid.tile([128, HW], BF, tag="attn_bc")
        for i in range(NJ):
            bc_ps = ps.tile([128, 512], F32, tag="big", name=f"bc_ps{i % 4}")
            nc.tensor.matmul(
                bc_ps[:], ones_row, attn_row[0:1, 512 * i:512 * (i + 1)]
            )
            if i % 2 == 0:
                nc.vector.tensor_copy(attn_bc[:, 512 * i:512 * (i + 1)], bc_ps[:])
            else:
                nc.scalar.copy(attn_bc[:, 512 * i:512 * (i + 1)], bc_ps[:])

        # ---- pass 2: context via fused (x * inv) * attn + row-reduce ----
        ttr_scr = [resid.tile([128, HW], BF, tag=f"ttr_scr{k}",
                               name=f"ttr_scr{k}") for k in range(2)]
        v0 = spool.tile([128, NJ], F32, tag="v0")
        for j in range(NJ):
            nc.vector.scalar_tensor_tensor(
                out=ttr_scr[j % 2][:],
                in0=x_sb[:, j, :],
                scalar=inv_bc[:],
                in1=attn_bc[:],
                op0=ALU.mult,
                op1=ALU.mult,
                accum_out=v0[:, j:j + 1],
            )

        # ---- gates ----
        # layer order: 0=(mul,r0) 1=(add,r0) 2=(mul,r1) 3=(add,r1)
        def gate_layer(lidx, v_bf, out_name):
            b1c = sm[:, 0 + 8 * lidx:8 + 8 * lidx]
            gc = sm[:, 32 + 8 * lidx:40 + 8 * lidx]
            bec = sm[:, 64 + 8 * lidx:72 + 8 * lidx]

            ps_h = ps.tile([128, NJ], F32, tag="tiny", bufs=4)
            for j in range(NJ):
                wt = wpool.tile([128, 1024], BF, tag="w1t")
                nc.sync.dma_start(wt[:], wg1_d[lidx, j])
                for m in range(NJ):
                    nc.tensor.matmul(
                        ps_h[:, m:m + 1],
                        wt[:, 128 * m:128 * (m + 1)],
                        v_bf[:, j:j + 1],
                        start=(j == 0 and m == 0),
                        stop=(j == NJ - 1 and m == NJ - 1),
                    )

            stats = spool.tile([128, 16], F32, tag="stats", bufs=2)
            nc.vector.tensor_add(stats[:, 0:8], ps_h[:], b1c)
            nc.vector.tensor_mul(stats[:, 8:16], stats[:, 0:8], stats[:, 0:8])

            ps_st = ps.tile([1, 16], F32, tag="tiny", bufs=4)
            nc.tensor.matmul(ps_st[:], ones_col, stats[:])

            w4 = spool.tile([1, 16], F32, tag="w4", bufs=2)
            # mean and mean-square per level (pairs of m-columns)
            nc.vector.reduce_sum(
                out=w4[0:1, 0:4],
                in_=ps_st[0:1, 0:8].rearrange("p (l t) -> p l t", t=2),
                axis=AX.X,
            )
            nc.vector.reduce_sum(
                out=w4[0:1, 4:8],
                in_=ps_st[0:1, 8:16].rearrange("p (l t) -> p l t", t=2),
                axis=AX.X,
            )
            nc.vector.tensor_scalar_mul(w4[0:1, 0:4], w4[0:1, 0:4], 1.0 / P)
            nc.vector.tensor_scalar_mul(w4[0:1, 4:8], w4[0:1, 4:8], 1.0 / P)
            nc.vector.tensor_mul(w4[0:1, 8:12], w4[0:1, 0:4], w4[0:1, 0:4])
            nc.vector.tensor_sub(w4[0:1, 4:8], w4[0:1, 4:8], w4[0:1, 8:12])
            nc.vector.tensor_scalar_add(w4[0:1, 4:8], w4[0:1, 4:8], EPS)
            nc.scalar.activation(w4[0:1, 4:8], w4[0:1, 4:8], AF.Sqrt)
            nc.vector.reciprocal(w4[0:1, 8:12], w4[0:1, 4:8])

            brow = spool.tile([1, 16], F32, tag="brow", bufs=2)
            bview = brow[0:1, 0:8].rearrange("p (l t) -> p t l", t=2)
            iview = brow[0:1, 8:16].rearrange("p (l t) -> p t l", t=2)
            for t in range(2):
                nc.vector.tensor_copy(bview[:, t, :], w4[0:1, 0:4])
                nc.vector.tensor_copy(iview[:, t, :], w4[0:1, 8:12])

            ps_bc = ps.tile([128, 16], F32, tag="tiny", bufs=4)
            nc.tensor.matmul(ps_bc[:], ones_row, brow[:])
            bc = spool.tile([128, 16], F32, tag="bc", bufs=2)
            nc.vector.tensor_copy(bc[:], ps_bc[:])

            hn = spool.tile([128, NJ], F32, tag="hn", bufs=2)
            nc.vector.tensor_sub(hn[:], stats[:, 0:8], bc[:, 0:8])
            nc.vector.tensor_mul(hn[:], hn[:], bc[:, 8:16])
            nc.vector.tensor_mul(hn[:], hn[:], gc)
            nc.vector.tensor_add(hn[:], hn[:], bec)
            hn_bf = spool.tile([128, NJ], BF, tag="hnbf", bufs=2)
            nc.scalar.activation(hn_bf[:], hn[:], AF.Relu)

            w2t = w2pool.tile([128, 2048], BF, tag="w2t")
            nc.sync.dma_start(w2t[:], wg2_d[lidx])
            ps_z = ps.tile([128, NJ], F32, tag="tiny", bufs=4)
            for lv in range(4):
                for kc in range(2):
                    for clc in range(2):
                        off = lv * 512 + kc * 256 + clc * 128
                        nc.tensor.matmul(
                            ps_z[:, 2 * lv + clc:2 * lv + clc + 1],
                            w2t[:, off:off + 128],
                            hn_bf[:, 2 * lv + kc:2 * lv + kc + 1],
                            start=(lv == 0 and kc == 0 and clc == 0),
                            stop=(lv == 3 and kc == 1 and clc == 1),
                        )
            zb = spool.tile([128, NJ], F32, tag=out_name)
            b2c = sm[:, 96 + 8 * lidx:104 + 8 * lidx]
            nc.vector.tensor_add(zb[:], ps_z[:], b2c)
            return zb

        def cast_bf(src, tag):
            t = spool.tile([128, NJ], BF, tag=tag)
            nc.vector.tensor_copy(t[:], src[:])
            return t

        v0_bf = cast_bf(v0, "v0bf")
        z_mul0 = gate_layer(0, v0_bf, "zmul0")
        z_add0 = gate_layer(1, v0_bf, "zadd0")

        vmul = spool.tile([128, NJ], F32, tag="vmul")
        nc.scalar.activation(vmul[:], z_mul0[:], AF.Sigmoid)
        vadd = z_add0

        z_mul1 = gate_layer(2, cast_bf(vmul, "vmbf"), "zmul1")
        z_add1 = gate_layer(3, cast_bf(vadd, "vabf"), "zadd1")

        mm_f = spool.tile([128, NJ], F32, tag="mmf")
        nc.scalar.activation(mm_f[:], z_mul1[:], AF.Sigmoid)
        nc.vector.tensor_add(mm_f[:], mm_f[:], vmul[:])
        ma_f = spool.tile([128, NJ], F32, tag="maf")
        nc.vector.tensor_add(ma_f[:], z_add1[:], vadd[:])

        # ---- pass 3: output ----
        addsum = spool.tile([128, 2], F32, tag="addsum")
        nc.vector.reduce_sum(
            out=addsum[:],
            in_=ma_f[:].rearrange("p (l t) -> p t l", t=2),
            axis=AX.X,
        )
        diags = []
        for js in range(NJ):
            dt_ = dpool.tile([128, 128], BF, tag=f"diag{js}", name=f"diag{js}")
            nc.vector.tensor_scalar_mul(dt_[:], rhsi[:], mm_f[:, js:js + 1])
            diags.append(dt_)

        for jj in range(2):
            for nch in range(NJ):
                ps_o = ps.tile([128, 512], F32, tag="big")
                for lv in range(4):
                    js = 2 * lv + jj
                    nc.tensor.matmul(
                        ps_o[:],
                        diags[js][:],
                        x_sb[:, js, 512 * nch:512 * (nch + 1)],
                        start=(lv == 0), stop=(lv == 3),
                    )
                stg = stpool.tile([128, 512], F32, tag="stg")
                nc.scalar.activation(
                    stg[:], ps_o[:], AF.Identity,
                    bias=addsum[:, jj:jj + 1], scale=1.0,
                )
                nc.sync.dma_start(
                    out_d[128 * jj:128 * (jj + 1), 512 * nch:512 * (nch + 1)],
                    stg[:],
                )

    nc.compile()
    return nc


def _pack_inputs(x0, x1, x2, x3, wm, bm,
                 add_W1, add_b1, add_g, add_be, add_W2, add_b2,
                 mul_W1, mul_b1, mul_g, mul_be, mul_W2, mul_b2):
    bf = ml_dtypes.bfloat16
    f32 = np.float32

    # shared (same for all cores)
    rhsi = np.eye(128, dtype=bf)
    wmc = np.asarray(wm, f32).reshape(NJ, 128).T.astype(bf).copy()
    cf32 = np.zeros((128, 256), f32)
    cf32[:, 0:128] = np.eye(128, dtype=f32)
    cf32[:, 128:256] = 1.0

    # gate weights, layer order: (mul,0) (add,0) (mul,1) (add,1)
    W1s = [mul_W1[0], add_W1[0], mul_W1[1], add_W1[1]]
    W2s = [mul_W2[0], add_W2[0], mul_W2[1], add_W2[1]]
    b1s = [mul_b1[0], add_b1[0], mul_b1[1], add_b1[1]]
    gs = [mul_g[0], add_g[0], mul_g[1], add_g[1]]
    bes = [mul_be[0], add_be[0], mul_be[1], add_be[1]]
    b2s = [mul_b2[0], add_b2[0], mul_b2[1], add_b2[1]]

    wg1 = np.zeros((4, NJ, 128, 1024), bf)
    wg2 = np.zeros((4, 128, 2048), bf)
    sm = np.zeros((128, 128), f32)
    for li in range(4):
        w1 = np.asarray(W1s[li], f32).reshape(C, C)       # [lp, c]
        # wg1[li, j, p, 128m+q] = w1[128m+q, 128j+p]
        t = w1.reshape(NJ, 128, NJ, 128)                   # [m, q, j, p]
        wg1[li] = t.transpose(2, 3, 0, 1).reshape(NJ, 128, 1024).astype(bf)
        w2 = np.asarray(W2s[li], f32)                      # [l, cl, pp]
        # wg2[li, p, l*512+kc*256+clc*128+q] = w2[l, 128clc+q, 128kc+p]
        t2 = w2.reshape(4, 2, 128, 2, 128)                 # [l, clc, q, kc, p]
        wg2[li] = t2.transpose(4, 0, 3, 1, 2).reshape(128, 2048).astype(bf)
        sm[:, 8 * li:8 * li + 8] = np.asarray(b1s[li], f32).reshape(C).reshape(NJ, 128).T
        sm[:, 32 + 8 * li:40 + 8 * li] = np.asarray(gs[li], f32).reshape(C).reshape(NJ, 128).T
        sm[:, 64 + 8 * li:72 + 8 * li] = np.asarray(bes[li], f32).reshape(C).reshape(NJ, 128).T
        b2 = np.asarray(b2s[li], f32)                      # [l, cl]
        sm[:, 96 + 8 * li:104 + 8 * li] = (
            b2.reshape(4, 2, 128).transpose(2, 0, 1).reshape(128, 8)
        )

    shared = dict(rhsi=rhsi, wmc=wmc, cf32=cf32, wg1=wg1, wg2=wg2, smalls=sm)

    in_maps = []
    xs = [np.asarray(a, f32) for a in (x0, x1, x2, x3)]
    for b in range(B):
        xc = np.concatenate(
            [a[b].reshape(CL, HW) for a in xs], axis=0
        ).astype(bf)
        in_maps.append({"x": xc, **shared})
    return in_maps


def kernel(**inputs):
    from concourse.bass_utils import run_bass_kernel_spmd

    if "nc" not in _CACHE:
        _CACHE["nc"] = _build_nc()
    nc = _CACHE["nc"]

    in_maps = _pack_inputs(**inputs)
    res = run_bass_kernel_spmd(nc, in_maps, list(range(NCORES)))
    _CACHE["last_results"] = res
    out = np.stack(
        [res.results[b]["out"].reshape(CL, H, W) for b in range(B)]
    ).astype(np.float32)
    return out



# revision 8
# speedup vs baseline: 1.5234x; 1.5234x over previous
# Trainium2 Bass kernel for the ContextBlock problem.
#
# Reference computation (per sample b):
#   xc    = concat(x0..x3)            [C=1024, HW=4096]
#   attn  = softmax(wm @ xc)          [HW]
#   ctx   = xc @ attn                 [C]
#   mul   = residual-gated MLP stack (sigmoid branch)   [C]
#   add   = residual-gated MLP stack (linear branch)    [C]
#   out   = sum_l (x_l * mul_l + add_l)                 [CL=256, HW]
#
# Distribution: data-parallel over batch, one sample per NeuronCore (B=8).
# No collectives required.
#
# Per-core dataflow (v2, pipelined):
#   x arrives in 4 column-blocks of 1024; per block: PE computes logits,
#   scalar exps them (unnormalized: the softmax scale cancels through the
#   gates' LayerNorm), PE broadcasts e across partitions, DVE+Scalar
#   accumulate u[c] = sum_n x[c,n] e[n].
#   Gates: both branches (mul/add) fused per repeat; weight-stationary
#   matvecs on PE; LN stats via gpsimd partition_all_reduce.
#   Pass3: out chunks split PE (diag-matmul) / Scalar+DVE (scalar chains).

import numpy as np
import ml_dtypes
from contextlib import ExitStack

import concourse.bass as bass
import concourse.bacc as bacc
import concourse.mybir as mybir
import concourse.tile as tile

BF = mybir.dt.bfloat16
F32 = mybir.dt.float32
AF = mybir.ActivationFunctionType
ALU = mybir.AluOpType
AX = mybir.AxisListType

B, L, CL, H, W = 8, 4, 256, 64, 64
C = L * CL          # 1024
HW = H * W          # 4096
P = C // 4          # 256
R = 2
EPS = 1e-5
NJ = C // 128       # 8   c-slabs
NBLK = 4            # x column blocks of 1024
BLKW = HW // NBLK   # 1024
NCORES = 8

_CACHE = {}


def _build_nc():
    import concourse.bass_isa as bass_isa

    nc = bacc.Bacc()

    x_d = nc.dram_tensor("x", [C, HW], BF, kind="ExternalInput")
    wmc_d = nc.dram_tensor("wmc", [128, NJ], BF, kind="ExternalInput")
    rhsi_d = nc.dram_tensor("rhsi", [128, 128], BF, kind="ExternalInput")
    onesr_d = nc.dram_tensor("onesr", [1, 128], BF, kind="ExternalInput")
    sm_d = nc.dram_tensor("smalls", [128, 128], F32, kind="ExternalInput")
    wg1_d = nc.dram_tensor("wg1", [R, NJ, 128, 2048], BF, kind="ExternalInput")
    wg2_d = nc.dram_tensor("wg2", [R, 128, 4096], BF, kind="ExternalInput")
    out_d = nc.dram_tensor("out", [CL, HW], F32, kind="ExternalOutput")

    with tile.TileContext(nc) as tc, ExitStack() as ctx:
        resid = ctx.enter_context(tc.tile_pool(name="resid", bufs=1))
        spool = ctx.enter_context(tc.tile_pool(name="spool", bufs=1))
        stpool = ctx.enter_context(tc.tile_pool(name="stage", bufs=6))
        apool = ctx.enter_context(tc.tile_pool(name="accp", bufs=2))

        # ---- resident tiles + loads (all weights resident; no recycling) --
        wmc = resid.tile([128, NJ], BF, tag="wmc")
        rhsi = resid.tile([128, 128], BF, tag="rhsi")
        onesr = resid.tile([1, 128], BF, tag="onesr")
        sm = resid.tile([128, 128], F32, tag="sm")
        x_sb = resid.tile([128, NJ, HW], BF, tag="x")
        wg1 = resid.tile([128, R, NJ, 2048], BF, tag="wg1")
        wg2 = resid.tile([128, R, 4096], BF, tag="wg2")

        nc.sync.dma_start(wmc[:], wmc_d[:])
        nc.sync.dma_start(rhsi[:], rhsi_d[:])
        nc.sync.dma_start(onesr[:], onesr_d[:])
        nc.sync.dma_start(sm[:], sm_d[:])
        for blk in range(NBLK):
            cols = slice(BLKW * blk, BLKW * (blk + 1))
            for j in range(NJ):
                nc.sync.dma_start(x_sb[:, j, cols], x_d[128 * j:128 * (j + 1), cols])
        for r in range(R):
            for j in range(NJ):
                nc.sync.dma_start(wg1[:, r, j, :], wg1_d[r, j])
            nc.sync.dma_start(wg2[:, r, :], wg2_d[r])

        # ---- activation-table warmup (Exp/Sqrt/Sigmoid/Identity) ----
        warm = spool.tile([1, 8], F32, tag="warm")
        nc.vector.memset(warm[:], 0.25)
        nc.scalar.activation(warm[0:1, 4:5], warm[0:1, 0:1], AF.Exp)
        nc.scalar.activation(warm[0:1, 5:6], warm[0:1, 1:2], AF.Sqrt)
        nc.scalar.activation(warm[0:1, 6:7], warm[0:1, 2:3], AF.Sigmoid)
        nc.scalar.activation(warm[0:1, 7:8], warm[0:1, 3:4], AF.Identity)

        # ---- phase A: logits -> exp -> u accumulation, per column block ---
        e_row = spool.tile([1, HW], BF, tag="e_row")
        e_bc = spool.tile([128, NBLK, BLKW], BF, tag="e_bc")
        scr_v = spool.tile([128, 2, BLKW], BF, tag="scr_v")
        prod = spool.tile([128, 3, BLKW], BF, tag="prod")
        scr_s = spool.tile([128, 2, BLKW], BF, tag="scr_s")
        u_parts = spool.tile([128, NJ * NBLK], F32, tag="u_parts")

        with tc.tile_pool(name="psA", bufs=2, space=bass.MemorySpace.PSUM) as psA:
            for blk in range(NBLK):
                cols = slice(BLKW * blk, BLKW * (blk + 1))
                lg = psA.tile([1, BLKW], F32, tag="lg")
                for j in range(NJ):
                    for h in range(2):
                        nc.tensor.matmul(
                            lg[0:1, 512 * h:512 * (h + 1)],
                            wmc[:, j:j + 1],
                            x_sb[:, j, BLKW * blk + 512 * h:BLKW * blk + 512 * (h + 1)],
                            start=(j == 0), stop=(j == NJ - 1),
                        )
                nc.scalar.activation(e_row[0:1, cols], lg[:], AF.Exp)
                for h in range(2):
                    bc_ps = psA.tile([128, 512], F32, tag="bc")
                    nc.tensor.matmul(
                        bc_ps[:], onesr[:],
                        e_row[0:1, BLKW * blk + 512 * h:BLKW * blk + 512 * (h + 1)],
                    )
                    nc.scalar.copy(
                        e_bc[:, blk, 512 * h:512 * (h + 1)], bc_ps[:]
                    )
                # u accumulation: slabs 0-4 fused STT on DVE;
                # slabs 5-7: 2x tensor_tensor on DVE + accum on Scalar
                for j in range(5):
                    nc.vector.scalar_tensor_tensor(
                        out=scr_v[:, blk % 2, :],
                        in0=x_sb[:, j, cols],
                        scalar=1.0,
                        in1=e_bc[:, blk, :],
                        op0=ALU.bypass,
                        op1=ALU.mult,
                        accum_out=u_parts[:, j * NBLK + blk:j * NBLK + blk + 1],
                    )
                for j in range(5, NJ):
                    nc.vector.tensor_mul(
                        prod[:, j - 5, :], x_sb[:, j, cols], e_bc[:, blk, :]
                    )
                    nc.scalar.activation(
                        scr_s[:, blk % 2, :], prod[:, j - 5, :], AF.Identity,
                        accum_out=u_parts[:, j * NBLK + blk:j * NBLK + blk + 1],
                    )

        v0 = spool.tile([128, NJ], F32, tag="v0")
        nc.vector.reduce_sum(
            out=v0[:],
            in_=u_parts[:].rearrange("p (j b) -> p j b", b=NBLK),
            axis=AX.X,
        )
        v0_bf = spool.tile([128, NJ], BF, tag="v0bf")
        nc.vector.tensor_copy(v0_bf[:], v0[:])

        # ---- gates: both branches fused per repeat ----
        # t-col layout: t = br*8 + 2*lv + half
        ps = ctx.enter_context(
            tc.tile_pool(name="psG", bufs=2, space=bass.MemorySpace.PSUM)
        )

        def gate_repeat(r, moving):
            b1c = sm[:, 0 + 16 * r:16 + 16 * r]
            gc = sm[:, 32 + 16 * r:48 + 16 * r]
            bec = sm[:, 64 + 16 * r:80 + 16 * r]
            b2c = sm[:, 96 + 16 * r:112 + 16 * r]

            ps_h = ps.tile([128, 16], F32, tag="ps_h")
            for j in range(NJ):
                for t in range(16):
                    nc.tensor.matmul(
                        ps_h[:, t:t + 1],
                        wg1[:, r, j, 128 * t:128 * (t + 1)],
                        moving(j, t // 8),
                        start=(j == 0 and t == 0),
                        stop=(j == NJ - 1 and t == 15),
                    )

            stats = spool.tile([128, 32], F32, tag="stats", bufs=2)
            nc.vector.tensor_add(stats[:, 0:16], ps_h[:], b1c)
            nc.vector.tensor_mul(stats[:, 16:32], stats[:, 0:16], stats[:, 0:16])

            allred = spool.tile([128, 32], F32, tag="allred", bufs=2)
            nc.gpsimd.partition_all_reduce(
                allred[:], stats[:], channels=128, reduce_op=bass_isa.ReduceOp.add
            )

            # per-group mean / var / rstd, replicated on all partitions
            gm = spool.tile([128, 16], F32, tag="gm", bufs=2)
            nc.vector.reduce_sum(
                out=gm[:],
                in_=allred[:].rearrange("p (g h) -> p g h", h=2),
                axis=AX.X,
            )
            nc.vector.tensor_scalar_mul(gm[:], gm[:], 1.0 / P)
            nbc = spool.tile([128, 16], F32, tag="nbc", bufs=2)
            nc.vector.tensor_mul(nbc[:, 0:8], gm[:, 0:8], gm[:, 0:8])
            nc.vector.tensor_sub(nbc[:, 8:16], gm[:, 8:16], nbc[:, 0:8])
            nc.vector.tensor_scalar_add(nbc[:, 8:16], nbc[:, 8:16], EPS)
            nc.scalar.activation(nbc[:, 0:8], nbc[:, 8:16], AF.Sqrt)
            nc.vector.reciprocal(nbc[:, 8:16], nbc[:, 0:8])
            # expand group scalars to per-t columns (t = 2g + half)
            mu_t = spool.tile([128, 32], F32, tag="mu_t", bufs=2)
            mtv = mu_t[:, 0:16].rearrange("p (g h) -> p h g", h=2)
            rtv = mu_t[:, 16:32].rearrange("p (g h) -> p h g", h=2)
            for hh in range(2):
                nc.vector.tensor_copy(mtv[:, hh, :], gm[:, 0:8])
                nc.vector.tensor_copy(rtv[:, hh, :], nbc[:, 8:16])

            hn = spool.tile([128, 16], F32, tag="hn", bufs=2)
            nc.vector.tensor_sub(hn[:], stats[:, 0:16], mu_t[:, 0:16])
            nc.vector.tensor_mul(hn[:], hn[:], mu_t[:, 16:32])
            nc.vector.tensor_mul(hn[:], hn[:], gc)
            nc.vector.tensor_add(hn[:], hn[:], bec)
            hn_bf = spool.tile([128, 16], BF, tag="hnbf", bufs=2)
            nc.vector.tensor_scalar_max(hn_bf[:], hn[:], 0.0)

            ps_z = ps.tile([128, 16], F32, tag="ps_z")
            nblks = 0
            for br in range(2):
                for lv in range(4):
                    for clc in range(2):
                        tcol = br * 8 + 2 * lv + clc
                        for kc in range(2):
                            off = (((br * 4 + lv) * 2 + clc) * 2 + kc) * 128
                            nc.tensor.matmul(
                                ps_z[:, tcol:tcol + 1],
                                wg2[:, r, off:off + 128],
                                hn_bf[:, br * 8 + 2 * lv + kc:br * 8 + 2 * lv + kc + 1],
                                start=(nblks == 0),
                                stop=(nblks == 31),
                            )
                            nblks += 1
            zb = spool.tile([128, 16], F32, tag=f"zb{r}")
            nc.vector.tensor_add(zb[:], ps_z[:], b2c)
            return zb

        zb0 = gate_repeat(0, lambda j, br: v0_bf[:, j:j + 1])

        vmul0 = spool.tile([128, NJ], F32, tag="vmul0")
        nc.scalar.activation(vmul0[:], zb0[:, 0:8], AF.Sigmoid)
        v1_bf = spool.tile([128, 16], BF, tag="v1bf")
        nc.vector.tensor_copy(v1_bf[:, 0:8], vmul0[:])
        nc.vector.tensor_copy(v1_bf[:, 8:16], zb0[:, 8:16])

        zb1 = gate_repeat(1, lambda j, br: v1_bf[:, br * 8 + j:br * 8 + j + 1])

        mm_f = spool.tile([128, NJ], F32, tag="mmf")
        nc.scalar.activation(mm_f[:], zb1[:, 0:8], AF.Sigmoid)
        nc.vector.tensor_add(mm_f[:], mm_f[:], vmul0[:])
        ma_f = spool.tile([128, NJ], F32, tag="maf")
        nc.vector.tensor_add(ma_f[:], zb1[:, 8:16], zb0[:, 8:16])

        # ---- pass 3 ----
        addsum = spool.tile([128, 2], F32, tag="addsum")
        nc.vector.reduce_sum(
            out=addsum[:],
            in_=ma_f[:].rearrange("p (l t) -> p t l", t=2),
            axis=AX.X,
        )
        dpool = ctx.enter_context(tc.tile_pool(name="diag", bufs=1))
        diags = []
        for js in range(NJ):
            dt_ = dpool.tile([128, 128], BF, tag=f"diag{js}", name=f"diag{js}")
            nc.vector.tensor_scalar_mul(dt_[:], rhsi[:], mm_f[:, js:js + 1])
            diags.append(dt_)

        with tc.tile_pool(name="psO", bufs=4, space=bass.MemorySpace.PSUM) as psO:
            for nch in range(NJ):
                for jj in range(2):
                    cols = slice(512 * nch, 512 * (nch + 1))
                    stg = stpool.tile([128, 512], F32, tag="stg")
                    if nch < 4:
                        # PE cells
                        ps_o = psO.tile([128, 512], F32, tag="big")
                        for lv in range(4):
                            js = 2 * lv + jj
                            nc.tensor.matmul(
                                ps_o[:], diags[js][:], x_sb[:, js, cols],
                                start=(lv == 0), stop=(lv == 3),
                            )
                        nc.scalar.activation(
                            stg[:], ps_o[:], AF.Identity,
                            bias=addsum[:, jj:jj + 1], scale=1.0,
                        )
                    else:
                        # Scalar does term 0 (+bias); DVE chains terms 1-3
                        acc = apool.tile([128, 2, 512], F32, tag="acc")
                        nc.scalar.activation(
                            acc[:, 0, :], x_sb[:, jj, cols], AF.Identity,
                            bias=addsum[:, jj:jj + 1],
                            scale=mm_f[:, jj:jj + 1],
                        )
                        for lv in range(1, 4):
                            js = 2 * lv + jj
                            nc.vector.scalar_tensor_tensor(
                                out=(stg[:] if lv == 3 else acc[:, lv % 2, :]),
                                in0=x_sb[:, js, cols],
                                scalar=mm_f[:, js:js + 1],
                                in1=acc[:, (lv - 1) % 2, :],
                                op0=ALU.mult, op1=ALU.add,
                            )
                    nc.sync.dma_start(
                        out_d[128 * jj:128 * (jj + 1), cols], stg[:],
                    )

    nc.compile()
    return nc


def _pack_inputs(x0, x1, x2, x3, wm, bm,
                 add_W1, add_b1, add_g, add_be, add_W2, add_b2,
                 mul_W1, mul_b1, mul_g, mul_be, mul_W2, mul_b2):
    bf = ml_dtypes.bfloat16
    f32 = np.float32

    wmc = np.asarray(wm, f32).reshape(NJ, 128).T.astype(bf).copy()
    rhsi = np.eye(128, dtype=bf)
    onesr = np.ones((1, 128), bf)

    wg1 = np.zeros((R, NJ, 128, 2048), bf)
    wg2 = np.zeros((R, 128, 4096), bf)
    sm = np.zeros((128, 128), f32)
    for r in range(R):
        for br, (W1, W2, b1, g, be, b2) in enumerate([
            (mul_W1[r], mul_W2[r], mul_b1[r], mul_g[r], mul_be[r], mul_b2[r]),
            (add_W1[r], add_W2[r], add_b1[r], add_g[r], add_be[r], add_b2[r]),
        ]):
            w1 = np.asarray(W1, f32).reshape(C, C)       # [lp, c]
            t1 = w1.reshape(NJ, 128, NJ, 128)             # [m, p', j, q]
            t1 = t1.transpose(2, 3, 0, 1).reshape(NJ, 128, 1024)
            wg1[r, :, :, 1024 * br:1024 * (br + 1)] = t1.astype(bf)

            w2 = np.asarray(W2, f32)                      # [lv, cl, p]
            t2 = w2.reshape(4, 2, 128, 2, 128)            # [lv, clc, cl', kc, q]
            t2 = t2.transpose(4, 0, 1, 3, 2).reshape(128, 2048)
            wg2[r, :, 2048 * br:2048 * (br + 1)] = t2.astype(bf)

            for arr, base in ((b1, 0), (g, 32), (be, 64)):
                a = np.asarray(arr, f32).reshape(C).reshape(NJ, 128).T
                sm[:, base + 16 * r + 8 * br: base + 16 * r + 8 * br + 8] = a
            b2a = np.asarray(b2, f32).reshape(4, 2, 128).transpose(2, 0, 1).reshape(128, 8)
            sm[:, 96 + 16 * r + 8 * br: 96 + 16 * r + 8 * br + 8] = b2a

    shared = dict(wmc=wmc, rhsi=rhsi, onesr=onesr, smalls=sm, wg1=wg1, wg2=wg2)

    in_maps = []
    xs = [np.asarray(a, f32) for a in (x0, x1, x2, x3)]
    for b in range(B):
        xc = np.concatenate(
            [a[b].reshape(CL, HW) for a in xs], axis=0
        ).astype(bf)
        in_maps.append({"x": xc, **shared})
    return in_maps


def kernel(**inputs):
    from concourse.bass_utils import run_bass_kernel_spmd

    if "nc" not in _CACHE:
        _CACHE["nc"] = _build_nc()
    nc = _CACHE["nc"]

    in_maps = _pack_inputs(**inputs)
    res = run_bass_kernel_spmd(nc, in_maps, list(range(NCORES)))
    _CACHE["last_results"] = res
    out = np.stack(
        [res.results[b]["out"].reshape(CL, H, W) for b in range(B)]
    ).astype(np.float32)
    return out
